# revision 10
# baseline (speedup 1.0000x reference)
"""Trainium2 Bass kernel for the PlaneElement kinematic-wave step.

Contract: kernel(**inputs) takes the FULL (unsharded) numpy inputs and
returns the full output -- here 4 scalars:
    (outflow_q, infil_rate_element, infil_depth_element, max_cfl)
as a float32 array of shape (4,).

Strategy:
  - Shard the 4M-node axis contiguously across 8 NeuronCores.
  - Each core gets a [128, 4099] f32 tile: partition p holds its 4096
    owned nodes plus a 3-element stencil halo (2 left, 1 right), with the
    halo baked in on the host (as_strided) -> no device halo exchange.
  - On-device math (per core, all in "SF = A/WID" units):
      SF      = relu(alpha*d + beta)            [ScalarE]
      infil t = min(a0 + a1*d, d + b0)          [DVE STT, fused sum accum]
      slope   = minmod via clamp identity       [DVE]
      SFface  = SF + 0.5*minmod                 [DVE STT]
      flux'   = SFface * exp(2/3*(lnAs-lnwp) + ln(r*m))   [ScalarE ln/exp + DVE]
      SFnext  = relu(SF - dflux')               [ScalarE]
      g2      = lnAs2 - lnwp2, fused max accum  [DVE TTR]
    max(vel) = m*exp(2/3*max g2) recovered on host (exp is monotone).
  - Tiny per-core partials ([128, 13]) are combined on the host; the
    outlet discharge and the two inlet-boundary nodes are computed
    exactly on the host in f64.
"""

import math

import numpy as np

N = 4_194_304
EPS = 1e-8
NCORES = 8
P = 128
F = 4096          # owned elements per partition
C = P * F         # owned elements per core
W = F + 3         # tile width incl. 3 halo columns
NCH = 4           # free-dim chunks for pipelining
CF = F // NCH     # owned columns per chunk
OUTC = 3 * NCH + 1

_prog_cache = {}


def _manning_q_np(A, WID, SS1, SS2, MAN, SL):
    h = A / WID
    wp = WID + h * (math.sqrt(1.0 + SS1 * SS1) + math.sqrt(1.0 + SS2 * SS2))
    A_safe = max(A, EPS)
    return A * (A_safe / wp) ** (2.0 / 3.0) * math.sqrt(SL) / MAN


def _build_program(consts):
    import concourse.bacc as bass
    import concourse.mybir as mybir
    from concourse.tile import TileContext

    (alpha, beta, a0, a1, b0, sconst, wid, ln_rm) = consts
    f32 = mybir.dt.float32
    Alu = mybir.AluOpType
    Act = mybir.ActivationFunctionType

    nc = bass.Bacc("TRN2", target_bir_lowering=False, debug=False,
                   num_devices=NCORES)
    d_in = nc.dram_tensor("d", [P, W], f32, kind="ExternalInput")
    o_out = nc.dram_tensor("out", [P, OUTC], f32, kind="ExternalOutput")

    # register activation-bias constants (bias must be a const AP)
    for i, val in enumerate({float(beta), float(EPS), float(wid),
                             float(ln_rm)}):
        if (f32, val) in nc.const_aps.aps:
            continue
        ct = nc.alloc_sbuf_tensor(f"constb-{i}", [P, 1], f32)
        nc.gpsimd.memset(ct.ap(), val)
        nc.const_aps.aps[(f32, val)] = ct.ap()
    nc.all_engine_barrier()

    with TileContext(nc) as tc:
        with tc.tile_pool(name="pool", bufs=2) as pool:
            out_tile = pool.tile([P, OUTC], f32, bufs=1)
            for c in range(NCH):
                o = c * CF
                L = CF + 3  # chunk input width

                dd = pool.tile([P, L], f32)
                nc.sync.dma_start(out=dd[:], in_=d_in[:, o:o + L])

                # surface depth after rain + infiltration (exact identity)
                SF = pool.tile([P, L], f32)
                nc.scalar.activation(SF[:], dd[:], Act.Relu,
                                     bias=beta, scale=alpha)

                # infiltration depth t = min(a0 + a1*d, d + b0), fused sum
                u = pool.tile([P, CF], f32)
                nc.vector.tensor_scalar(u[:], dd[:, 2:2 + CF], a1, a0,
                                        Alu.mult, Alu.add)
                t = pool.tile([P, CF], f32)
                nc.vector.scalar_tensor_tensor(
                    t[:], dd[:, 2:2 + CF], b0, u[:], Alu.add, Alu.min,
                    accum_out=out_tile[:, c:c + 1])

                # MUSCL limiter: minmod(x,y) = clamp(y, min(x,0), max(x,0))
                dSF = pool.tile([P, L - 1], f32)
                nc.vector.tensor_sub(dSF[:], SF[:, 1:L], SF[:, 0:L - 1])
                xm = pool.tile([P, L - 1], f32)
                nc.vector.tensor_scalar_min(xm[:], dSF[:], 0.0)
                xp = pool.tile([P, L - 1], f32)
                nc.vector.tensor_scalar_max(xp[:], dSF[:], 0.0)
                c1 = pool.tile([P, L - 2], f32)
                nc.vector.tensor_tensor(c1[:], dSF[:, 1:L - 1],
                                        xm[:, 0:L - 2], Alu.max)
                c2 = pool.tile([P, L - 2], f32)
                nc.vector.tensor_tensor(c2[:], c1[:], xp[:, 0:L - 2], Alu.min)
                SFf = pool.tile([P, L - 2], f32)
                nc.vector.scalar_tensor_tensor(
                    SFf[:], c2[:], 0.5, SF[:, 1:L - 1], Alu.mult, Alu.add)

                # Manning flux on face states, in log space
                lnAs = pool.tile([P, L - 2], f32)
                nc.scalar.activation(lnAs[:], SFf[:], Act.Ln,
                                     bias=EPS, scale=wid)
                lnwp = pool.tile([P, L - 2], f32)
                nc.scalar.activation(lnwp[:], SFf[:], Act.Ln,
                                     bias=wid, scale=sconst)
                g1 = pool.tile([P, L - 2], f32)
                nc.vector.tensor_sub(g1[:], lnAs[:], lnwp[:])
                pw = pool.tile([P, L - 2], f32)
                nc.scalar.activation(pw[:], g1[:], Act.Exp,
                                     bias=ln_rm, scale=2.0 / 3.0)
                fx = pool.tile([P, L - 2], f32)
                nc.vector.tensor_mul(fx[:], SFf[:], pw[:])

                # conservative update
                fd = pool.tile([P, CF], f32)
                nc.vector.tensor_sub(fd[:], fx[:, 1:CF + 1], fx[:, 0:CF])
                s2 = pool.tile([P, CF], f32)
                nc.vector.tensor_sub(s2[:], SF[:, 2:2 + CF], fd[:])
                SFn = pool.tile([P, CF], f32)
                nc.scalar.activation(SFn[:], s2[:], Act.Relu)

                # CFL: g2 = ln(A_safe) - ln(wp) on updated state, fused max
                lnA2 = pool.tile([P, CF], f32)
                nc.scalar.activation(lnA2[:], SFn[:], Act.Ln,
                                     bias=EPS, scale=wid)
                lnw2 = pool.tile([P, CF], f32)
                nc.scalar.activation(lnw2[:], SFn[:], Act.Ln,
                                     bias=wid, scale=sconst)
                g2 = pool.tile([P, CF], f32)
                nc.vector.tensor_sub(g2[:], lnA2[:], lnw2[:])
                nc.vector.tensor_reduce(
                    out_tile[:, 4 + c:5 + c], g2[:, 2:CF],
                    mybir.AxisListType.X, Alu.max)
                nc.vector.tensor_reduce(
                    out_tile[:, 8 + c:9 + c], g2[:, 0:2],
                    mybir.AxisListType.X, Alu.max)

                if c == NCH - 1:
                    nc.vector.tensor_copy(out_tile[:, 12:13],
                                          SFn[:, CF - 1:CF])

            nc.sync.dma_start(out=o_out[:, :], in_=out_tile[:])

    nc.compile()
    return nc


def _run_device(shards, consts, trace=False):
    from concourse.bass_utils import run_bass_kernel_spmd

    key = tuple(consts)
    if key not in _prog_cache:
        _prog_cache[key] = _build_program(consts)
    nc = _prog_cache[key]
    in_maps = [{"d": shards[i]} for i in range(NCORES)]
    res = run_bass_kernel_spmd(nc, in_maps, core_ids=list(range(NCORES)),
                               trace=trace)
    return res


def kernel(depth, rain_rate, dt, cum_rain, theta_current, F_cumulative,
           WID, SS1, SS2, MAN, SL, dx, Ks, psi, theta_s, _trace=False,
           _return_results=False):
    depth = np.asarray(depth, np.float32)
    rain_rate = float(rain_rate)
    dt = float(dt)
    theta_current = float(theta_current)
    F_cumulative = float(F_cumulative)
    WID = float(WID)
    SS1 = float(SS1)
    SS2 = float(SS2)
    MAN = float(MAN)
    SL = float(SL)
    dx = float(dx)
    Ks = float(Ks)
    psi = float(psi)
    theta_s = float(theta_s)

    # host-folded scalar coefficients (f64)
    dtheta = max(theta_s - theta_current, 0.0)
    F_safe = max(F_cumulative, 1e-6)
    a1 = Ks * dt / F_safe                       # fp*dt = a0 + a1*d
    a0 = Ks * dt * (1.0 + psi * dtheta / F_safe)
    b0 = rain_rate * dt                         # avail = d + b0
    alpha = 1.0 - a1                            # surf = relu(alpha*d + beta)
    beta = b0 - a0
    sconst = math.sqrt(1.0 + SS1 * SS1) + math.sqrt(1.0 + SS2 * SS2)
    m = math.sqrt(SL) / MAN
    r = dt / dx
    # In SF = A/WID units: SF_next = relu(SF - (f_i - f_{i-1})) with
    #   f = (r/WID)*q(A_face) = r*m*SFface*ratio^(2/3),
    #   ratio = max(WID*SFface, EPS)/(WID + sconst*SFface)
    # so lnAs = ln(WID*SFface + EPS), lnwp = ln(WID + sconst*SFface) and
    # the exp bias is exactly ln(r*m).
    ln_rm = math.log(max(r * m, 1e-38))
    consts = (alpha, beta, a0, a1, b0, sconst, WID, ln_rm)

    # --- host shard prep: [128, 4099] per core with baked halo ---
    padded = np.empty(N + 3, np.float32)
    padded[2:2 + N] = depth
    padded[0:2] = 0.0          # left ghosts (nodes 0,1 host-corrected)
    padded[N + 2] = depth[-1]  # right ghost replicates -> slope[N-1] = 0
    shards = []
    for k in range(NCORES):
        base = padded[k * C:k * C + C + 3]
        sh = np.lib.stride_tricks.as_strided(
            base, shape=(P, W), strides=(F * 4, 4)).copy()
        shards.append(np.ascontiguousarray(sh))

    res = _run_device(shards, consts, trace=_trace)
    outs = [res.results[i]["out"] for i in range(NCORES)]

    # --- host combine ---
    sum_t = np.float64(0.0)
    for k in range(NCORES):
        sum_t += np.sum(outs[k][:, 0:NCH].astype(np.float64))
    infil_depth = sum_t / N
    infil_rate = infil_depth / dt

    g2max = -np.inf
    for k in range(NCORES):
        g2max = max(g2max, float(outs[k][:, NCH:2 * NCH].max()))
        edge = outs[k][:, 2 * NCH:3 * NCH].astype(np.float64).copy()
        if k == 0:
            edge[0, 0] = -np.inf  # polluted inlet nodes 0,1
        g2max = max(g2max, float(edge.max()))
    max_vel = m * math.exp((2.0 / 3.0) * g2max) if m > 0 else 0.0

    # exact inlet nodes 0 and 1 on host (f64), matching reference BCs
    d0, d1, d2 = (float(depth[0]), float(depth[1]), float(depth[2]))

    def _surf(d):
        t = min(a0 + a1 * d, d + b0)
        return max(d + b0 - t, 0.0)

    A0, A1, A2 = (WID * _surf(d0), WID * _surf(d1), WID * _surf(d2))
    # slope[0] = 0; slope[1] = minmod(A1-A0, A2-A1)
    x, y = A1 - A0, A2 - A1
    mm1 = min(max(y, min(x, 0.0)), max(x, 0.0))
    Af0 = A0
    Af1 = A1 + 0.5 * mm1
    q0 = _manning_q_np(Af0, WID, SS1, SS2, MAN, SL)
    q1 = _manning_q_np(Af1, WID, SS1, SS2, MAN, SL)
    An0 = max(A0 - r * (q0 - 0.0), 0.0)
    An1 = max(A1 - r * (q1 - q0), 0.0)
    for An in (An0, An1):
        Q = _manning_q_np(An, WID, SS1, SS2, MAN, SL)
        max_vel = max(max_vel, Q / max(An, EPS))

    max_cfl = max_vel * dt / dx

    # outlet discharge from the device's last updated state
    sfl = float(outs[NCORES - 1][P - 1, 3 * NCH])
    A_last = WID * sfl
    outflow_q = _manning_q_np(A_last, WID, SS1, SS2, MAN, SL)

    out = np.array([outflow_q, infil_rate, infil_depth, max_cfl], np.float32)
    if _return_results:
        return out, res
    return out
